# revision 6
# baseline (speedup 1.0000x reference)
"""CLAHE (nn_EqualizeClahe) Trainium2 Bass kernel.

kernel(x): x (8,3,1024,1024) fp32 in [0,1) -> same-shape output.
8 NeuronCores data parallel: core i processes image i (3 channels).

Per channel (1024x1024, 8x8 grid of 128x128 tiles):
  prep:  b = trunc(x*256) (exact: x*256 is an exact fp32 product for
         x = k*2^-24), split b = b16 + lo nibbles; idx = trunc(x*255)
         (matches reference's int32 truncation of the identical product).
  hist:  per tile, 256 bins as a 16x16 outer product accumulated on the
         TensorEngine: for each of the 128 pixel-columns c of a tile,
         psum[16,16] += OH_c^T @ OL_c, with OH/OL 16-wide one-hots of
         hi/lo built by DVE is_equal slabs (bf16).
  lut:   clip at 2560; the uniform redistribution is applied analytically
         after the cumsum: C~[i] = C[i] + (i+1)*base + min(i+1, residual).
         Cumsum = shift-add doubling within the 16 lo-bins + triangular
         matmul across the 16 hi-rows.  All integer arithmetic is exact
         in fp32 (sums <= 16384; 255/16384 is a power-of-two scaled int).
  apply: the channel LUT is exported to DRAM expanded per 64-px cell
         column pair (lutx[r, jx, bin, gx], gx = left/right tile of the
         cell).  Per 128-row band, two broadcast DMAs pull the top and
         bottom grid-row slabs onto the partitions; DVE pre-blends the
         vertical weight wy(row) into a per-partition (per-row) table
         rowLUT[p] = wy*top + (1-wy)*bot.  One indirect_copy per
         512-column chunk gathers d=2 (left/right) values for all
         pixels: out[p, (s*16+j)*2+g] = rowLUT[p][idx of pixel
         (16*(p//16)+j, col s)], valid where j == p%16 (per-partition
         tables).  16 Activation-engine copies extract the valid
         diagonal, then a 3-op DVE horizontal blend and store.

Self-contained: only needs /opt/trn_rl_repo (concourse) + numpy.
"""
import sys

for _p in ("/opt/trn_rl_repo",):
    if _p not in sys.path:
        sys.path.insert(0, _p)

import dataclasses
from contextlib import ExitStack

import numpy as np

import concourse.bass as bass
import concourse.mybir as mybir
import concourse.tile as tile
from concourse.bass_utils import run_bass_kernel_spmd

FP32 = mybir.dt.float32
BF16 = mybir.dt.bfloat16
I32 = mybir.dt.int32
U16 = mybir.dt.uint16
OP = mybir.AluOpType
AF = mybir.ActivationFunctionType

H = W = 1024
CH = 3
NB = 256
TS = 128
PIX = TS * TS
MAXV = 2560.0
SCALE = float((NB - 1) / PIX)

_CACHE = {}


# ----------------------------------------------------------------- helpers
def _bcast(ap, dim_counts):
    """Append step-0 dims (broadcast) to an AP: dim_counts = [n, ...]"""
    new = list(ap.ap) + [[0, n] for n in dim_counts]
    return dataclasses.replace(ap, ap=new)


def _interp_coords(n_tiles, tile_size, length):
    half = tile_size // 2
    pos = np.arange(length)
    j = pos // half
    p = pos % half
    r0 = np.clip((j - 1) // 2, 0, n_tiles - 1)
    r1 = np.clip(r0 + 1, 0, n_tiles - 1)
    denom = np.float32(2 * half - 1)
    w = np.where(j % 2 == 1, (2 * half - 1) - p, (half - 1) - p).astype(np.float32) / denom
    w = np.where(j == 0, np.float32(1.0), w).astype(np.float32)
    return r0, r1, w


def _host_consts():
    c = {}
    # lhsT[k, j] = 1 iff k < j  (prefix sums over the 16 hi-rows)
    c["ltri"] = np.triu(np.ones((16, 16), np.float32), 1)
    c["ones16"] = np.ones((16, 1), np.float32)
    c["iota1"] = (np.arange(256, dtype=np.float32).reshape(16, 16) + 1.0)
    r0, r1, wy = _interp_coords(8, TS, H)
    c0_, c1_, wx = _interp_coords(8, TS, W)
    c["wy"] = np.ascontiguousarray(wy.reshape(8, 128).T)           # [128, 8]
    c["wx2"] = np.ascontiguousarray(
        np.broadcast_to((wx / np.float32(255.0))[None, :], (128, W))).astype(np.float32)
    jx = (np.arange(W) // 64).astype(np.float32)
    c["gb2"] = np.ascontiguousarray(
        np.broadcast_to((jx * 512.0)[None, :], (128, W))).astype(np.float32)
    return c


# ----------------------------------------------------------------- kernel IR
def _emit(nc, tc, ctx, x_in, y_out, K):
    r0f, r1f, _ = _interp_coords(8, TS, H)
    c0f, c1f, _ = _interp_coords(8, TS, W)
    c0s = c0f[::64]   # per 64-px cell: left tile column (16 cells)
    c1s = c1f[::64]   # per 64-px cell: right tile column

    pool = ctx.enter_context(tc.tile_pool(name="main", bufs=1))
    pspool = ctx.enter_context(tc.tile_pool(name="ps", bufs=4, space="PSUM"))
    ps1pool = ctx.enter_context(tc.tile_pool(name="ps1", bufs=1, space="PSUM"))

    # constants
    ltri = pool.tile([16, 16], FP32, tag="ltri")
    nc.sync.dma_start(ltri[:], K["ltri"].ap())
    ones16 = pool.tile([16, 1], FP32, tag="ones16")
    nc.sync.dma_start(ones16[:], K["ones16"].ap())
    iota1 = pool.tile([16, 16], FP32, tag="iota1")
    nc.sync.dma_start(iota1[:], K["iota1"].ap())
    wyt = pool.tile([128, 8], FP32, tag="wy")
    nc.sync.dma_start(wyt[:], K["wy"].ap())
    wx2t = pool.tile([128, W], FP32, tag="wx2")
    nc.sync.dma_start(wx2t[:], K["wx2"].ap())
    gb2 = pool.tile([128, W], FP32, tag="gb2")
    nc.sync.dma_start(gb2[:], K["gb2"].ap())

    lutx = K["lutx"]  # dram [CH, 65536] bf16: ((r*16+jx)*256+i)*2+gx

    def trunc_prep(ch, a, scale):
        """load band a, return fp32 tile holding trunc(x*scale) (exact)."""
        xband = pool.tile([128, W], FP32, tag="xband")
        nc.sync.dma_start(xband[:], x_in[ch, a * 128:(a + 1) * 128, :])
        scrA = pool.tile([128, W], FP32, tag="scrA")
        nc.scalar.activation(scrA[:], xband[:], AF.Copy, scale=float(scale))
        scrI = pool.tile([128, W], I32, tag="scrI")
        nc.vector.tensor_copy(scrI[:], scrA[:])
        scrB = pool.tile([128, W], FP32, tag="scrB")
        nc.vector.tensor_copy(scrB[:], scrI[:])
        fx = pool.tile([128, W], FP32, tag="fx")
        nc.vector.tensor_tensor(fx[:], scrB[:], scrA[:], op=OP.is_gt)
        nc.vector.tensor_tensor(scrB[:], scrB[:], fx[:], op=OP.subtract)
        return scrB  # trunc(x*scale), fp32

    for ch in range(CH):
        # ---------------- histogram ----------------
        hsb = pool.tile([16, 64 * 16], FP32, tag="hsb")
        for a in range(8):
            b = trunc_prep(ch, a, 256.0)          # bin index 0..255 (fp32)
            # hi = floor(b/16) via RNE(b/16 - 0.46875) (b integer, exact)
            scrI2 = pool.tile([128, W], I32, tag="scrI")
            nc.vector.tensor_scalar(scrI2[:], b[:], 0.0625, -0.46875,
                                    op0=OP.mult, op1=OP.add)
            fx2 = pool.tile([128, W], FP32, tag="fx")
            nc.vector.tensor_copy(fx2[:], scrI2[:])
            b16 = pool.tile([128, W], BF16, tag="b16")
            nc.vector.tensor_scalar(b16[:], fx2[:], 16.0, None, op0=OP.mult)
            lo = pool.tile([128, W], BF16, tag="lo")
            nc.vector.tensor_tensor(lo[:], b[:], b16[:], op=OP.subtract)

            ohh = pool.tile([128, 16 * W], BF16, tag="big1")
            ohl = pool.tile([128, 16 * W], BF16, tag="big2")
            for j in range(16):
                nc.vector.tensor_scalar(ohh[:, j * W:(j + 1) * W], b16[:],
                                        float(16 * j), None, op0=OP.is_equal)
                nc.vector.tensor_scalar(ohl[:, j * W:(j + 1) * W], lo[:],
                                        float(j), None, op0=OP.is_equal)
            oh3 = ohh[:].rearrange("p (j x) -> p j x", j=16)
            ol3 = ohl[:].rearrange("p (j x) -> p j x", j=16)
            for t in range(8):
                ps = pspool.tile([16, 16], FP32, tag="hps")
                for cc in range(128):
                    col = t * 128 + cc
                    nc.tensor.matmul(ps[:], oh3[:, :, col], ol3[:, :, col],
                                     start=(cc == 0), stop=(cc == 127))
                ti = a * 8 + t
                nc.vector.tensor_scalar(hsb[:, ti * 16:(ti + 1) * 16], ps[:],
                                        MAXV, None, op0=OP.min)

        # ---------------- LUT build [16, (t,k)] ----------------
        r1t = pool.tile([16, 64 * 16], FP32, tag="r1")
        r2t = pool.tile([16, 64 * 16], FP32, tag="r2")

        def shift_add(dst, src, s):
            nc.vector.tensor_copy(dst[:], src[:])
            d3 = dst[:].rearrange("p (t k) -> p t k", k=16)[:, :, s:]
            s3 = src[:].rearrange("p (t k) -> p t k", k=16)[:, :, :16 - s]
            nc.vector.tensor_tensor(d3, d3, s3, op=OP.add)

        shift_add(r1t, hsb, 1)
        shift_add(r2t, r1t, 2)
        shift_add(r1t, r2t, 4)
        shift_add(r2t, r1t, 8)

        rt = r2t[:].rearrange("p (t k) -> p t k", k=16)[:, :, 15]
        pre_ps = ps1pool.tile([16, 64], FP32, tag="pre")
        nc.tensor.matmul(pre_ps[:], ltri[:], rt, start=True, stop=True)
        tot_ps = ps1pool.tile([1, 64], FP32, tag="tot")
        nc.tensor.matmul(tot_ps[:], ones16[:], rt, start=True, stop=True)
        tot = pool.tile([1, 64], FP32, tag="tot")
        nc.vector.tensor_copy(tot[:], tot_ps[:])
        o1 = pool.tile([1, 16], FP32, tag="o1")
        nc.vector.memset(o1[:], 1.0)
        tot16_ps = ps1pool.tile([16, 64], FP32, tag="tot16")
        nc.tensor.matmul(tot16_ps[:], o1[:], tot[:], start=True, stop=True)

        clip16 = pool.tile([16, 64], FP32, tag="clip16")
        nc.vector.tensor_scalar(clip16[:], tot16_ps[:], -1.0, 16384.0,
                                op0=OP.mult, op1=OP.add)
        basev = pool.tile([16, 64], FP32, tag="basev")
        nc.vector.tensor_scalar(basev[:], clip16[:], 1.0 / 256.0, None, op0=OP.mult)
        ri = pool.tile([16, 64], I32, tag="ri")
        nc.vector.tensor_copy(ri[:], basev[:])
        rf = pool.tile([16, 64], FP32, tag="rf")
        nc.vector.tensor_copy(rf[:], ri[:])
        resid = pool.tile([16, 64], FP32, tag="resid")
        nc.vector.tensor_tensor(resid[:], rf[:], basev[:], op=OP.is_gt)
        nc.vector.tensor_tensor(basev[:], rf[:], resid[:], op=OP.subtract)  # base=floor
        nc.vector.scalar_tensor_tensor(resid[:], basev[:], -256.0, clip16[:],
                                       op0=OP.mult, op1=OP.add)             # resid

        ct = r2t[:].rearrange("p (t k) -> p t k", k=16)
        pre = pool.tile([16, 64], FP32, tag="presb")
        nc.vector.tensor_copy(pre[:], pre_ps[:])
        nc.vector.tensor_tensor(ct, ct, _bcast(pre[:], [16]), op=OP.add)
        tmp = pool.tile([16, 64 * 16], FP32, tag="tmpc")
        tmp3 = tmp[:].rearrange("p (t k) -> p t k", k=16)
        iota_b = dataclasses.replace(iota1[:], ap=[iota1[:].ap[0], [0, 64], iota1[:].ap[1]])
        nc.vector.tensor_tensor(tmp3, iota_b, _bcast(basev[:], [16]), op=OP.mult)
        nc.vector.tensor_tensor(ct, ct, tmp3, op=OP.add)
        nc.vector.tensor_tensor(tmp3, iota_b, _bcast(resid[:], [16]), op=OP.min)
        nc.vector.tensor_tensor(ct, ct, tmp3, op=OP.add)

        nc.vector.tensor_scalar(r2t[:], r2t[:], SCALE, None, op0=OP.mult)
        li = pool.tile([16, 64 * 16], I32, tag="li")
        nc.vector.tensor_copy(li[:], r2t[:])
        nc.vector.tensor_copy(r1t[:], li[:])
        lfx = pool.tile([16, 64 * 16], FP32, tag="lfx")
        nc.vector.tensor_tensor(lfx[:], r1t[:], r2t[:], op=OP.is_gt)
        lutb = pool.tile([16, 64 * 16], BF16, tag="lutb")
        nc.vector.tensor_tensor(lutb[:], r1t[:], lfx[:], op=OP.subtract)

        # ---------------- export expanded LUT to DRAM ----------------
        # lutbI[hi, ((r*16+jx)*16 + lo)*2 + gx] = lutb[hi, (r*8+c(jx,gx))*16+lo]
        # cell->tile map: jx=2m+1+e (m=0..6): c0=m, c1=m+1; edges jx=0,15.
        def _ap(t, off, dims):
            a = t[:]
            return dataclasses.replace(a, offset=a.offset + off,
                                       ap=[list(a.ap[0])] + [list(d) for d in dims])

        lutbI = pool.tile([16, 4096], BF16, tag="lutbI")
        for gx in range(2):
            for e in range(2):
                # jx = 1+e, 3+e, ..., 13+e  (m = 0..6), c = m + gx
                nc.vector.tensor_copy(
                    _ap(lutbI, (1 + e) * 32 + gx,
                        [[512, 8], [64, 7], [2, 16]]),
                    _ap(lutb, gx * 16,
                        [[128, 8], [16, 7], [1, 16]]))
            # edges: jx=0 -> c = gx*1 (c0=0/c1=1); jx=15 -> c=7
            cstep = 7 - gx
            nc.vector.tensor_copy(
                _ap(lutbI, gx, [[512, 8], [15 * 32, 2], [2, 16]]),
                _ap(lutb, gx * 16, [[128, 8], [cstep * 16, 2], [1, 16]]))
        # lutx[ch, ((r*16+jx)*256 + hi*16+lo)*2 + gx] = lutbI[hi, ...]
        base_ap = lutx.ap()
        dst = dataclasses.replace(
            base_ap, offset=base_ap.offset + ch * 65536,
            ap=[[32, 16], [512, 128], [1, 32]])
        nc.sync.dma_start(dst, lutbI[:].rearrange("p (a b) -> p a b", b=32))

        # ---------------- apply ----------------
        for a in range(8):
            # per-half top/bottom LUT slabs -> topbot [128, 2*8192] bf16
            topbot = pool.tile([128, 2 * 8192], BF16, tag="big3")
            for h in range(2):
                rT = int(r0f[a * 128 + h * 64])
                rB = int(r1f[a * 128 + h * 64])
                src = dataclasses.replace(
                    base_ap,
                    offset=base_ap.offset + ch * 65536 + rT * 8192,
                    ap=[[0, 64], [(rB - rT) * 8192, 2], [1, 8192]])
                nc.sync.dma_start(topbot[h * 64:(h + 1) * 64, :], src)
            # rowLUT[p] = wy(p)*top + (1-wy(p))*bot   (per-row table)
            rowlut = pool.tile([128, 8192], BF16, tag="rowlut")
            nc.vector.tensor_tensor(rowlut[:], topbot[:, :8192],
                                    topbot[:, 8192:], op=OP.subtract)
            nc.vector.scalar_tensor_tensor(rowlut[:], rowlut[:], wyt[:, a:a + 1],
                                           topbot[:, 8192:],
                                           op0=OP.mult, op1=OP.add)

            # flat gather idx: (jx*256 + trunc(x*255))*2, uint16
            idxf = trunc_prep(ch, a, 255.0)
            flat = pool.tile([128, W], U16, tag="flat")
            nc.vector.scalar_tensor_tensor(flat[:], idxf[:], 2.0, gb2[:],
                                           op0=OP.mult, op1=OP.add)

            gpx = pool.tile([128, 2048], BF16, tag="gpx")
            data3 = rowlut[:].rearrange("p (i d) -> p i d", d=2)
            for c in range(2):
                gout = pool.tile([128, 16384], BF16, tag=("big1", "big2")[c])
                nc.gpsimd.indirect_copy(
                    gout[:].rearrange("p (i d) -> p i d", d=2),
                    data3, flat[:, c * 512:(c + 1) * 512], True)
                # extract valid diagonal j == p%16 (only DMA may stride
                # partitions; engines require partition step 1)
                g4 = gout[:].rearrange("p (s j g) -> p s j g", j=16, g=2)
                o4 = gpx[:].rearrange("p (c s g) -> p c s g", c=2, g=2)
                for j in range(16):
                    nc.sync.dma_start(o4[j::16, c, :, :], g4[j::16, :, j, :])

            # horizontal blend: res = ((g0-g1)*wx + g1)/255
            g2v = gpx[:].rearrange("p (x g) -> p x g", g=2)
            bd = pool.tile([128, W], FP32, tag="bd")
            nc.vector.tensor_tensor(bd[:], g2v[:, :, 0], g2v[:, :, 1],
                                    op=OP.subtract)
            bt = pool.tile([128, W], FP32, tag="bt")
            nc.vector.tensor_tensor(bt[:], bd[:], wx2t[:], op=OP.mult)
            res = pool.tile([128, W], FP32, tag="res")
            nc.vector.scalar_tensor_tensor(res[:], g2v[:, :, 1],
                                           float(np.float32(1.0) / np.float32(255.0)),
                                           bt[:], op0=OP.mult, op1=OP.add)
            nc.sync.dma_start(y_out[ch, a * 128:(a + 1) * 128, :], res[:])


def _apply_tile_patch():
    """This walrus build rejects >2 sync waits on one instruction; split the
    TileContext exit drain's waits into individual nops."""
    def _patched(self, tick_clock, wait_clock):
        nc = self.nc
        probe = nc.sync.nop()
        wait_clock.add_sem_waits(probe.ins,
                                 tile.ScopedClock({None: tick_clock.global_clock}))
        si = probe.ins.sync_info
        waits = list(si.on_wait) if si and si.on_wait else []
        if len(waits) > 1:
            probe.ins.sync_info = mybir.SyncInfo(on_wait=[waits[0]], on_update=[])
            for w in waits[1:]:
                extra = nc.sync.nop()
                extra.ins.sync_info = mybir.SyncInfo(on_wait=[w], on_update=[])
        nc.sync.drain()
        nc.all_engine_barrier()
        assert self.sems is not None
        popped = nc._tile_sem_poison_stack.pop()
        assert popped is self._sem_poison
        nc.clear_and_free_semaphores(list(self.sems.allocated().values()))
        nc.all_engine_barrier()
    tile.TileContext._drain_and_barrier = _patched


def _split_waits(nc, maxw=1):
    """This container's walrus rejects instructions with more than ~2 sem
    waits; hoist excess waits onto same-engine NoOps inserted just before."""
    import bass_rust
    counter = [0]
    for f in nc.m.functions:
        for blk in f.blocks:
            insts = blk.instructions
            out = []
            for ins in insts:
                si = ins.sync_info
                waits = list(si.on_wait) if si and si.on_wait else []
                if len(waits) > maxw:
                    keep = waits[:maxw]
                    extra = waits[maxw:]
                    for w in extra:
                        counter[0] += 1
                        nop = bass_rust.InstNoOp(
                            name=f"WSPLIT-{counter[0]}", engine=ins.engine,
                            ins=[], outs=[],
                            sync_info=mybir.SyncInfo(on_wait=[w], on_update=[]))
                        out.append(nop)
                    ins.sync_info = mybir.SyncInfo(
                        on_wait=keep, on_update=list(si.on_update or []))
                out.append(ins)
            blk.instructions = out


def build():
    if "nc" in _CACHE:
        return _CACHE["nc"]
    _apply_tile_patch()
    nc = bass.Bass("TRN2", target_bir_lowering=False, debug=False)
    x_in = nc.dram_tensor("x", [CH, H, W], FP32, kind="ExternalInput").ap()
    y_out = nc.dram_tensor("y", [CH, H, W], FP32, kind="ExternalOutput").ap()
    hk = _host_consts()
    K = {k: nc.inline_tensor(v, name=f"const_{k}") for k, v in hk.items()}
    K["lutx"] = nc.dram_tensor("lutx", [CH, 65536], BF16)
    with ExitStack() as ctx:
        tc = ctx.enter_context(tile.TileContext(nc))
        _emit(nc, tc, ctx, x_in, y_out, K)
    _split_waits(nc)
    _CACHE["nc"] = nc
    return nc


def kernel(x: np.ndarray) -> np.ndarray:
    x = np.ascontiguousarray(np.asarray(x, dtype=np.float32))
    assert x.shape == (8, CH, H, W), x.shape
    nc = build()
    in_maps = [{"x": x[i]} for i in range(8)]
    res = run_bass_kernel_spmd(nc, in_maps, list(range(8)))
    out = np.stack([res.results[i]["y"] for i in range(8)], axis=0)
    return out.astype(np.float32)


if __name__ == "__main__":
    x = np.random.rand(8, CH, H, W).astype(np.float32)
    y = kernel(x)
    print("ran:", y.shape, y.dtype)
